# revision 21
# baseline (speedup 1.0000x reference)
"""MemoryUnit prototype kernel for 8 Trainium2 NeuronCores.

Full inputs in, full outputs out. Data-parallel: batch 32 -> 4 samples/core.

Math (per sample, x flattened to xb [C=256, N=4096], tok = xb^T):
  l  = W @ xb                      [M=64, N]
  E1 = exp(l)                      (softmax-over-N denominators cancel in l2norm)
  proto_raw = E1 @ tok             [M, C]
  proto = proto_raw / |proto_raw|  (L2 over C)
  s2 = proto @ xb                  [M, N]
  E2 = exp(s2)                     (softmax-over-M denominators cancel)
  nq_raw = proto^T @ E2            [C, N]   -> host: nq_raw / |nq_raw| = output
  summax = sum_n log(max_m E2)     (for compact loss; argmax dot == max)
  dis = sum_triu relu(2*proto@proto^T - 1)
Host:
  compact = (sum(x^2) - 2*sum_b summax_b + B*N) / (B*N*C)
  dis_loss = mean_b(dis_b) * 2/(M*(M-1))
"""
import sys

sys.path.insert(0, "/opt/trn_rl_repo")

from contextlib import ExitStack

import numpy as np

import concourse.bacc as bacc
import concourse.tile as tile
from concourse import mybir
from concourse.bass_utils import run_bass_kernel_spmd
from concourse.tile import add_dep_helper

N_CORES = 8
B, C, H, Wd = 32, 256, 64, 64
N = H * Wd          # 4096
M = 64              # prototypes
BS = B // N_CORES   # 4 samples per core
NPAIR = BS // 2     # samples processed in pairs (pack 2x64 -> 128 partitions)
NCH = N // 128      # 32 chunks of 128 pixels
NJ = N // 512       # 8 chunks of 512 pixels

F32 = mybir.dt.float32
BF16 = mybir.dt.bfloat16

_CACHE = {}


def _build_nc(iters=1):
    nc = bacc.Bacc("TRN2", target_bir_lowering=False, debug=False,
                   num_devices=N_CORES)

    x = nc.dram_tensor("x", [BS, C, N], F32, kind="ExternalInput").ap()
    wt = nc.dram_tensor("wt", [2, 128, M], BF16, kind="ExternalInput").ap()
    ident = nc.dram_tensor("ident", [128, M], F32, kind="ExternalInput").ap()
    triu = nc.dram_tensor("triu", [M, M], F32, kind="ExternalInput").ap()
    ones64 = nc.dram_tensor("ones64", [M, 1], F32, kind="ExternalInput").ap()
    ones128 = nc.dram_tensor("ones128", [128, 1], F32, kind="ExternalInput").ap()

    out_nq = nc.dram_tensor("out_nq", [BS, 2, 128, N], F32,
                            kind="ExternalOutput").ap()
    scal = nc.dram_tensor("scal", [BS, 2], F32, kind="ExternalOutput").ap()

    with tile.TileContext(nc) as tc, ExitStack() as ctx:
        singles = ctx.enter_context(tc.tile_pool(name="singles", bufs=1))
        px = ctx.enter_context(tc.tile_pool(name="px", bufs=2))
        ptokp = ctx.enter_context(tc.tile_pool(name="ptokp", bufs=1))
        pe12 = ctx.enter_context(tc.tile_pool(name="pe12", bufs=2))
        pt = ctx.enter_context(tc.tile_pool(name="pt", bufs=1))
        mid = ctx.enter_context(tc.tile_pool(name="mid", bufs=2))
        small = ctx.enter_context(tc.tile_pool(name="small", bufs=2))
        stage = ctx.enter_context(tc.tile_pool(name="stage", bufs=4))
        ppA = ctx.enter_context(tc.tile_pool(name="ppA", bufs=2, space="PSUM"))
        ppB = ctx.enter_context(tc.tile_pool(name="ppB", bufs=1, space="PSUM"))
        ppC = ctx.enter_context(tc.tile_pool(name="ppC", bufs=2, space="PSUM"))
        ppD = ctx.enter_context(tc.tile_pool(name="ppD", bufs=2, space="PSUM"))
        ppS = ctx.enter_context(tc.tile_pool(name="ppS", bufs=1, space="PSUM"))

        # constants
        wt_sb = singles.tile([128, 2, M], BF16)
        for k in range(2):
            nc.sync.dma_start(out=wt_sb[:, k, :], in_=wt[k])
        ident_sb = singles.tile([128, M], F32)
        nc.sync.dma_start(out=ident_sb, in_=ident)
        triu_sb = singles.tile([M, M], F32)
        nc.sync.dma_start(out=triu_sb, in_=triu)
        ones64_sb = singles.tile([M, 1], F32)
        nc.sync.dma_start(out=ones64_sb, in_=ones64)
        ones128_sb = singles.tile([128, 1], F32)
        nc.sync.dma_start(out=ones128_sb, in_=ones128)
        scal_sb = singles.tile([BS, 2], F32)
        neg1 = singles.tile([M, 1], F32)
        nc.vector.memset(neg1, -1.0)
        disrow_all = singles.tile([M, BS], F32)
        lnacc_all = singles.tile([128, BS], F32)

        # Concurrent in-flight xbar transposes hard-hang the S2M xbar unit
        # (HW bug; Tile does not serialize these). Chain them explicitly.
        xbar_chain = []

        def xbar(out, in_):
            xi = nc.sync.dma_start(out=out, in_=in_, transpose=True)
            if xbar_chain:
                add_dep_helper(xi.ins, xbar_chain[-1].ins, sync=True,
                               reason="serialize xbar")
            xbar_chain.append(xi)
            return xi

        for p in [pp for _ in range(iters) for pp in range(NPAIR)]:
            bb = (2 * p, 2 * p + 1)

            # ---- load + cast f32->bf16 (SWDGE), then xbar-transpose to tok
            # xbp[:, 2*s+k, :] = x[b_s, k*128:(k+1)*128, :] in bf16
            xbp = px.tile([128, 4, N], BF16, tag="xbp")
            for s in range(2):
                for k in range(2):
                    nc.gpsimd.dma_start(out=xbp[:, 2 * s + k, :],
                                        in_=x[bb[s], k * 128:(k + 1) * 128, :])
            # tokp[:, s, i, :] = tok chunk i of sample s ([128 pix, 256 ch])
            tokp = ptokp.tile([128, 2, NCH, C], BF16, tag="tokp")
            for s in range(2):
                for k in range(2):
                    xbar(tokp[:, s, :, k * 128:(k + 1) * 128],
                         xbp[:, 2 * s + k, :])

            # ---- A: E1 = exp(W @ xb), pair-packed [128, N]
            e1 = pe12.tile([128, N], BF16, tag="e1")
            for j in range(NJ):
                psA = ppA.tile([128, 512], F32, tag="psA")
                for s in range(2):
                    for k in range(2):
                        nc.tensor.matmul(psA[s * 64:(s + 1) * 64, :],
                                         lhsT=wt_sb[:, k, :],
                                         rhs=xbp[:, 2 * s + k,
                                                 j * 512:(j + 1) * 512],
                                         start=(k == 0), stop=(k == 1))
                nc.scalar.activation(e1[:, j * 512:(j + 1) * 512], psA,
                                     mybir.ActivationFunctionType.Exp)

            # ---- E1^T via xbar: [128, NCH, 128] (cols s*64.. = sample s)
            e1t = pt.tile([128, NCH, 128], BF16, tag="e1t")
            xbar(e1t, e1)

            # ---- B: proto_raw = sum_n E1T . tok   (pair-packed [128, C])
            psB = ppB.tile([128, C], F32, tag="psB")
            for s in range(2):
                for i in range(NCH):
                    nc.tensor.matmul(psB[s * 64:(s + 1) * 64, :],
                                     lhsT=e1t[:, i, s * 64:(s + 1) * 64],
                                     rhs=tokp[:, s, i, :],
                                     start=(i == 0), stop=(i == NCH - 1))

            # ---- proto normalization (rsqrt via exp(-0.5*ln); one table set)
            scr = mid.tile([128, C], F32, tag="scr")
            p2 = small.tile([128, 1], F32, tag="p2")
            nc.scalar.activation(scr, psB,
                                 mybir.ActivationFunctionType.Square,
                                 accum_out=p2)
            lnp2 = small.tile([128, 1], F32, tag="lnp2")
            nc.scalar.activation(lnp2, p2, mybir.ActivationFunctionType.Ln)
            inv = small.tile([128, 1], F32, tag="inv")
            nc.scalar.activation(inv, lnp2, mybir.ActivationFunctionType.Exp,
                                 scale=-0.5)
            proto_f = mid.tile([128, C], F32, tag="proto_f")
            nc.scalar.activation(proto_f, psB,
                                 mybir.ActivationFunctionType.Copy, scale=inv)
            proto_b = mid.tile([128, C], BF16, tag="proto_b")
            nc.vector.tensor_copy(proto_b, proto_f)

            # ---- protoT per sample (f32 + bf16) via PE transpose
            ptT_f = mid.tile([128, 2, 2, M], F32, tag="ptT_f")
            ptT_b = mid.tile([128, 2, 2, M], BF16, tag="ptT_b")
            for s in range(2):
                for k in range(2):
                    psT = ppS.tile([128, M], F32, tag="psS")
                    nc.tensor.transpose(
                        psT, proto_f[s * 64:(s + 1) * 64,
                                     k * 128:(k + 1) * 128],
                        ident_sb[s * 64:(s + 1) * 64, :])
                    nc.vector.tensor_copy(ptT_f[:, s, k, :], psT)
                    nc.vector.tensor_copy(ptT_b[:, s, k, :], psT)

            # ---- Gp + dis per sample (f32, tiny)
            for s in range(2):
                psG = ppS.tile([128, M], F32, tag="psS")
                for k in range(2):
                    nc.tensor.matmul(psG[0:M, :], lhsT=ptT_f[:, s, k, :],
                                     rhs=ptT_f[:, s, k, :],
                                     start=(k == 0), stop=(k == 1))
                dis = small.tile([M, M], F32, tag="dis")
                nc.scalar.activation(dis, psG[0:M, :],
                                     mybir.ActivationFunctionType.Relu,
                                     bias=neg1, scale=2.0)
                scr64 = small.tile([M, M], F32, tag="scr64")
                nc.vector.tensor_mul(scr64, dis, triu_sb)
                nc.vector.reduce_sum(
                    out=disrow_all[:, bb[s]:bb[s] + 1], in_=scr64,
                    axis=mybir.AxisListType.X)

            # ---- C: E2 = exp(proto @ xb), pair-packed [128, N]
            e2 = pe12.tile([128, N], BF16, tag="e2")
            for j in range(NJ):
                psC = ppC.tile([128, 512], F32, tag="psC")
                for s in range(2):
                    for k in range(2):
                        nc.tensor.matmul(psC[s * 64:(s + 1) * 64, :],
                                         lhsT=ptT_b[:, s, k, :],
                                         rhs=xbp[:, 2 * s + k,
                                                 j * 512:(j + 1) * 512],
                                         start=(k == 0), stop=(k == 1))
                nc.scalar.activation(e2[:, j * 512:(j + 1) * 512], psC,
                                     mybir.ActivationFunctionType.Exp)

            # ---- E2^T via xbar + smax
            e2t = pt.tile([128, NCH, 128], BF16, tag="e2t")
            xbar(e2t, e2)
            for s in range(2):
                smx = small.tile([128, NCH], F32, tag="smx")
                nc.vector.tensor_reduce(
                    out=smx, in_=e2t[:, :, s * 64:(s + 1) * 64],
                    axis=mybir.AxisListType.X, op=mybir.AluOpType.max)
                lnsm = small.tile([128, NCH], F32, tag="lnsm")
                nc.scalar.activation(lnsm, smx,
                                     mybir.ActivationFunctionType.Ln,
                                     accum_out=lnacc_all[:, bb[s]:bb[s] + 1])

            # ---- D: nq_raw[c, n] = sum_m proto[m, c] E2[m, n]  -> DRAM
            for s in range(2):
                for cc in range(2):
                    for j in range(NJ):
                        psD = ppD.tile([128, 512], F32, tag="psD")
                        nc.tensor.matmul(
                            psD,
                            lhsT=proto_b[s * 64:(s + 1) * 64,
                                         cc * 128:(cc + 1) * 128],
                            rhs=e2[s * 64:(s + 1) * 64, j * 512:(j + 1) * 512],
                            start=True, stop=True)
                        st = stage.tile([128, 512], F32, tag="st")
                        if j % 2 == 0:
                            nc.vector.tensor_copy(st, psD)
                        else:
                            nc.scalar.copy(st, psD)
                        nc.sync.dma_start(
                            out=out_nq[bb[s], cc, :, j * 512:(j + 1) * 512],
                            in_=st)

        # ---- batched partition sums for the loss scalars
        psq = ppS.tile([128, M], F32, tag="psS")
        nc.tensor.matmul(psq[0:BS, 0:1], lhsT=disrow_all, rhs=ones64_sb,
                         start=True, stop=True)
        nc.vector.tensor_copy(scal_sb[:, 1:2], psq[0:BS, 0:1])
        psq2 = ppS.tile([128, M], F32, tag="psS")
        nc.tensor.matmul(psq2[0:BS, 0:1], lhsT=lnacc_all, rhs=ones128_sb,
                         start=True, stop=True)
        nc.vector.tensor_copy(scal_sb[:, 0:1], psq2[0:BS, 0:1])
        nc.sync.dma_start(out=scal, in_=scal_sb)

    nc.compile()
    return nc


def _get_nc(iters=1):
    key = ("nc", iters)
    if key not in _CACHE:
        _CACHE[key] = _build_nc(iters)
    return _CACHE[key]


def _host_constants():
    if "consts" not in _CACHE:
        ident = np.vstack([np.eye(M, dtype=np.float32)] * 2)
        triu = np.triu(np.ones((M, M), np.float32), k=1)
        ones64 = np.ones((M, 1), np.float32)
        ones128 = np.ones((128, 1), np.float32)
        _CACHE["consts"] = dict(ident=ident, triu=triu, ones64=ones64,
                                ones128=ones128)
    return _CACHE["consts"]


def kernel(x, W, label_batch=None, _want_time=False, _iters=1, **_ignored):
    import ml_dtypes
    x = np.asarray(x, dtype=np.float32)
    W = np.asarray(W, dtype=np.float32)
    nc = _get_nc(_iters)
    consts = _host_constants()

    wt = np.ascontiguousarray(
        W.T.reshape(2, 128, M)).astype(ml_dtypes.bfloat16)
    x_flat = x.reshape(B, C, N)

    in_maps = []
    for i in range(N_CORES):
        in_maps.append(dict(
            x=np.ascontiguousarray(x_flat[i * BS:(i + 1) * BS]),
            wt=wt, **consts))

    res = run_bass_kernel_spmd(nc, in_maps, list(range(N_CORES)))

    nq = np.concatenate(
        [res.results[i]["out_nq"].reshape(BS, C, N) for i in range(N_CORES)],
        axis=0)                                   # [B, C, N] unnormalized
    scal = np.stack([res.results[i]["scal"] for i in range(N_CORES)])  # [8,BS,2]
    summax = scal[:, :, 0].reshape(-1)            # [B]
    dis = scal[:, :, 1].reshape(-1)               # [B]

    # host: l2-normalize over C and reshape; losses
    nrm = np.sqrt(np.einsum("bcn,bcn->bn", nq, nq))
    np.maximum(nrm, 1e-12, out=nrm)
    out = (nq / nrm[:, None, :]).reshape(B, C, H, Wd).astype(np.float32)

    frob2 = float(np.dot(x.reshape(-1), x.reshape(-1)))
    compact = np.float32(
        (frob2 - 2.0 * float(summax.sum()) + B * N) / (B * N * C))
    dis_loss = np.float32(dis.mean() * 2.0 / (M * (M - 1)))

    if _want_time:
        return (out, compact, dis_loss), res
    return (out, compact, dis_loss)


# revision 26
# speedup vs baseline: 1.7424x; 1.7424x over previous
"""MemoryUnit prototype kernel for 8 Trainium2 NeuronCores.

Full inputs in, full outputs out. Data-parallel: batch 32 -> 4 samples/core.

Math (per sample, x flattened to xb [C=256, N=4096], tok = xb^T):
  l  = W @ xb                      [M=64, N]
  E1 = exp(l)                      (softmax-over-N denominators cancel in l2norm)
  proto_raw = E1 @ tok             [M, C]
  proto = proto_raw / |proto_raw|  (L2 over C)
  s2 = proto @ xb                  [M, N]
  E2 = exp(s2)                     (softmax-over-M denominators cancel)
  nq_raw = proto^T @ E2            [C, N]   -> host: nq_raw / |nq_raw| = output
  summax = sum_n log(max_m E2)     (for compact loss; argmax dot == max)
  dis = sum_triu relu(2*proto@proto^T - 1)
Host:
  compact = (sum(x^2) - 2*sum_b summax_b + B*N) / (B*N*C)
  dis_loss = mean_b(dis_b) * 2/(M*(M-1))
"""
import sys

sys.path.insert(0, "/opt/trn_rl_repo")

from contextlib import ExitStack

import numpy as np

import concourse.bacc as bacc
import concourse.tile as tile
from concourse import mybir
from concourse.bass_utils import run_bass_kernel_spmd
from concourse.tile import add_dep_helper

N_CORES = 8
B, C, H, Wd = 32, 256, 64, 64
N = H * Wd          # 4096
M = 64              # prototypes
BS = B // N_CORES   # 4 samples per core
NPAIR = BS // 2     # samples processed in pairs (pack 2x64 -> 128 partitions)
NCH = N // 128      # 32 chunks of 128 pixels
NJ = N // 512       # 8 chunks of 512 pixels

F32 = mybir.dt.float32
BF16 = mybir.dt.bfloat16

_CACHE = {}


def _build_nc(iters=1):
    nc = bacc.Bacc("TRN2", target_bir_lowering=False, debug=False,
                   num_devices=N_CORES)

    x = nc.dram_tensor("x", [BS, C, N], F32, kind="ExternalInput").ap()
    wt = nc.dram_tensor("wt", [2, 128, M], BF16, kind="ExternalInput").ap()
    ident = nc.dram_tensor("ident", [128, M], F32, kind="ExternalInput").ap()
    identb = nc.dram_tensor("identb", [128, 128], BF16,
                            kind="ExternalInput").ap()
    triu = nc.dram_tensor("triu", [M, M], F32, kind="ExternalInput").ap()
    ones64 = nc.dram_tensor("ones64", [M, 1], F32, kind="ExternalInput").ap()
    ones128 = nc.dram_tensor("ones128", [128, 1], F32, kind="ExternalInput").ap()

    out_nq = nc.dram_tensor("out_nq", [BS, 2, 128, N], F32,
                            kind="ExternalOutput").ap()
    scal = nc.dram_tensor("scal", [BS, 2], F32, kind="ExternalOutput").ap()

    with tile.TileContext(nc) as tc, ExitStack() as ctx:
        singles = ctx.enter_context(tc.tile_pool(name="singles", bufs=1))
        px = ctx.enter_context(tc.tile_pool(name="px", bufs=2))
        ptokp = ctx.enter_context(tc.tile_pool(name="ptokp", bufs=1))
        pe12 = ctx.enter_context(tc.tile_pool(name="pe12", bufs=2))
        pt = ctx.enter_context(tc.tile_pool(name="pt", bufs=1))
        mid = ctx.enter_context(tc.tile_pool(name="mid", bufs=2))
        small = ctx.enter_context(tc.tile_pool(name="small", bufs=2))
        stage = ctx.enter_context(tc.tile_pool(name="stage", bufs=4))
        ppA = ctx.enter_context(tc.tile_pool(name="ppA", bufs=2, space="PSUM"))
        ppB = ctx.enter_context(tc.tile_pool(name="ppB", bufs=1, space="PSUM"))
        ppC = ctx.enter_context(tc.tile_pool(name="ppC", bufs=2, space="PSUM"))
        ppD = ctx.enter_context(tc.tile_pool(name="ppD", bufs=2, space="PSUM"))
        ppS = ctx.enter_context(tc.tile_pool(name="ppS", bufs=1, space="PSUM"))

        # constants
        wt_sb = singles.tile([128, 2, M], BF16)
        for k in range(2):
            nc.sync.dma_start(out=wt_sb[:, k, :], in_=wt[k])
        ident_sb = singles.tile([128, M], F32)
        nc.sync.dma_start(out=ident_sb, in_=ident)
        identb_sb = singles.tile([128, 128], BF16)
        nc.sync.dma_start(out=identb_sb, in_=identb)
        triu_sb = singles.tile([M, M], F32)
        nc.sync.dma_start(out=triu_sb, in_=triu)
        ones64_sb = singles.tile([M, 1], F32)
        nc.sync.dma_start(out=ones64_sb, in_=ones64)
        ones128_sb = singles.tile([128, 1], F32)
        nc.sync.dma_start(out=ones128_sb, in_=ones128)
        scal_sb = singles.tile([BS, 2], F32)
        neg1 = singles.tile([M, 1], F32)
        nc.vector.memset(neg1, -1.0)
        disrow_all = singles.tile([M, BS], F32)
        lnacc_all = singles.tile([128, BS], F32)

        # Concurrent in-flight xbar transposes hard-hang the S2M xbar unit
        # (HW bug; Tile does not serialize these). Chain them explicitly.
        xbar_chain = []

        def xbar(out, in_):
            xi = nc.sync.dma_start(out=out, in_=in_, transpose=True)
            if xbar_chain:
                add_dep_helper(xi.ins, xbar_chain[-1].ins, sync=True,
                               reason="serialize xbar")
            xbar_chain.append(xi)
            return xi

        for p in [pp for _ in range(iters) for pp in range(NPAIR)]:
            bb = (2 * p, 2 * p + 1)

            # ---- load + cast f32->bf16 (SWDGE), then xbar-transpose to tok
            # xbp[:, 2*s+k, :] = x[b_s, k*128:(k+1)*128, :] in bf16
            xbp = px.tile([128, 4, N], BF16, tag="xbp")
            for s in range(2):
                for k in range(2):
                    nc.gpsimd.dma_start(out=xbp[:, 2 * s + k, :],
                                        in_=x[bb[s], k * 128:(k + 1) * 128, :])
            # tokp[:, s, i, :] = tok chunk i of sample s ([128 pix, 256 ch])
            tokp = ptokp.tile([128, 2, NCH, C], BF16, tag="tokp")
            for s in range(2):
                for k in range(2):
                    xbar(tokp[:, s, :, k * 128:(k + 1) * 128],
                         xbp[:, 2 * s + k, :])

            # ---- A: E1 = exp(W @ xb), pair-packed [128, N]
            e1 = pe12.tile([128, N], BF16, tag="e1")
            for j in range(NJ):
                psA = ppA.tile([128, 512], F32, tag="psA")
                for s in range(2):
                    for k in range(2):
                        nc.tensor.matmul(psA[s * 64:(s + 1) * 64, :],
                                         lhsT=wt_sb[:, k, :],
                                         rhs=xbp[:, 2 * s + k,
                                                 j * 512:(j + 1) * 512],
                                         start=(k == 0), stop=(k == 1))
                nc.scalar.activation(e1[:, j * 512:(j + 1) * 512], psA,
                                     mybir.ActivationFunctionType.Exp)

            # ---- E1^T via xbar: [128, NCH, 128] (cols s*64.. = sample s)
            e1t = pt.tile([128, NCH, 128], BF16, tag="e1t")
            xbar(e1t, e1)

            # ---- B: proto_raw = sum_n E1T . tok   (pair-packed [128, C])
            psB = ppB.tile([128, C], F32, tag="psB")
            for s in range(2):
                for i in range(NCH):
                    nc.tensor.matmul(psB[s * 64:(s + 1) * 64, :],
                                     lhsT=e1t[:, i, s * 64:(s + 1) * 64],
                                     rhs=tokp[:, s, i, :],
                                     start=(i == 0), stop=(i == NCH - 1))

            # ---- proto normalization (rsqrt via exp(-0.5*ln); one table set)
            scr = mid.tile([128, C], F32, tag="scr")
            p2 = small.tile([128, 1], F32, tag="p2")
            nc.scalar.activation(scr, psB,
                                 mybir.ActivationFunctionType.Square,
                                 accum_out=p2)
            lnp2 = small.tile([128, 1], F32, tag="lnp2")
            nc.scalar.activation(lnp2, p2, mybir.ActivationFunctionType.Ln)
            inv = small.tile([128, 1], F32, tag="inv")
            nc.scalar.activation(inv, lnp2, mybir.ActivationFunctionType.Exp,
                                 scale=-0.5)
            proto_f = mid.tile([128, C], F32, tag="proto_f")
            nc.scalar.activation(proto_f, psB,
                                 mybir.ActivationFunctionType.Copy, scale=inv)
            proto_b = mid.tile([128, C], BF16, tag="proto_b")
            nc.vector.tensor_copy(proto_b, proto_f)

            # ---- protoT per sample (f32 + bf16) via PE transpose
            ptT_f = mid.tile([128, 2, 2, M], F32, tag="ptT_f")
            ptT_b = mid.tile([128, 2, 2, M], BF16, tag="ptT_b")
            for s in range(2):
                for k in range(2):
                    psT = ppS.tile([128, M], F32, tag="psS")
                    nc.tensor.transpose(
                        psT, proto_f[s * 64:(s + 1) * 64,
                                     k * 128:(k + 1) * 128],
                        ident_sb[s * 64:(s + 1) * 64, :])
                    nc.vector.tensor_copy(ptT_f[:, s, k, :], psT)
                    nc.vector.tensor_copy(ptT_b[:, s, k, :], psT)

            # ---- Gp + dis per sample (f32, tiny)
            for s in range(2):
                psG = ppS.tile([128, M], F32, tag="psS")
                for k in range(2):
                    nc.tensor.matmul(psG[0:M, :], lhsT=ptT_f[:, s, k, :],
                                     rhs=ptT_f[:, s, k, :],
                                     start=(k == 0), stop=(k == 1))
                dis = small.tile([M, M], F32, tag="dis")
                nc.scalar.activation(dis, psG[0:M, :],
                                     mybir.ActivationFunctionType.Relu,
                                     bias=neg1, scale=2.0)
                scr64 = small.tile([M, M], F32, tag="scr64")
                nc.vector.tensor_mul(scr64, dis, triu_sb)
                nc.vector.reduce_sum(
                    out=disrow_all[:, bb[s]:bb[s] + 1], in_=scr64,
                    axis=mybir.AxisListType.X)

            # ---- C: E2 = exp(proto @ xb), pair-packed [128, N]
            e2 = pe12.tile([128, N], BF16, tag="e2")
            for j in range(NJ):
                psC = ppC.tile([128, 512], F32, tag="psC")
                for s in range(2):
                    for k in range(2):
                        nc.tensor.matmul(psC[s * 64:(s + 1) * 64, :],
                                         lhsT=ptT_b[:, s, k, :],
                                         rhs=xbp[:, 2 * s + k,
                                                 j * 512:(j + 1) * 512],
                                         start=(k == 0), stop=(k == 1))
                nc.scalar.activation(e2[:, j * 512:(j + 1) * 512], psC,
                                     mybir.ActivationFunctionType.Exp)

            # ---- smax via PE chunk transposes (E2 pair chunk [128m, 128n]
            # -> [128n, 128m]), DVE max over each sample's 64 m-columns
            smxall = small.tile([128, 2, NCH], F32, tag="smxall")
            for i in range(NCH):
                psTT = ppD.tile([128, 128], BF16, tag="psD")
                nc.tensor.transpose(psTT, e2[:, i * 128:(i + 1) * 128],
                                    identb_sb)
                nc.vector.tensor_reduce(
                    out=smxall[:, :, i],
                    in_=psTT.rearrange("p (s m) -> p s m", s=2),
                    axis=mybir.AxisListType.X, op=mybir.AluOpType.max)
            for s in range(2):
                lnsm = small.tile([128, NCH], F32, tag="lnsm")
                nc.scalar.activation(lnsm, smxall[:, s, :],
                                     mybir.ActivationFunctionType.Ln,
                                     accum_out=lnacc_all[:, bb[s]:bb[s] + 1])

            # ---- D: nq_raw[c, n] = sum_m proto[m, c] E2[m, n]  -> DRAM
            # two 512-chunks share one staging tile -> one 4KB-row DMA
            for s in range(2):
                for cc in range(2):
                    for j2 in range(NJ // 2):
                        st = stage.tile([128, 1024], F32, tag="st")
                        for h in range(2):
                            j = 2 * j2 + h
                            psD = ppD.tile([128, 512], F32, tag="psD")
                            nc.tensor.matmul(
                                psD,
                                lhsT=proto_b[s * 64:(s + 1) * 64,
                                             cc * 128:(cc + 1) * 128],
                                rhs=e2[s * 64:(s + 1) * 64,
                                       j * 512:(j + 1) * 512],
                                start=True, stop=True)
                            if j % 4 == 3:
                                nc.scalar.copy(st[:, h * 512:(h + 1) * 512],
                                               psD)
                            else:
                                nc.vector.tensor_copy(
                                    st[:, h * 512:(h + 1) * 512], psD)
                        nc.sync.dma_start(
                            out=out_nq[bb[s], cc, :,
                                       j2 * 1024:(j2 + 1) * 1024],
                            in_=st)

        # ---- batched partition sums for the loss scalars
        psq = ppS.tile([128, M], F32, tag="psS")
        nc.tensor.matmul(psq[0:BS, 0:1], lhsT=disrow_all, rhs=ones64_sb,
                         start=True, stop=True)
        nc.vector.tensor_copy(scal_sb[:, 1:2], psq[0:BS, 0:1])
        psq2 = ppS.tile([128, M], F32, tag="psS")
        nc.tensor.matmul(psq2[0:BS, 0:1], lhsT=lnacc_all, rhs=ones128_sb,
                         start=True, stop=True)
        nc.vector.tensor_copy(scal_sb[:, 0:1], psq2[0:BS, 0:1])
        nc.sync.dma_start(out=scal, in_=scal_sb)

    nc.compile()
    return nc


def _get_nc(iters=1):
    key = ("nc", iters)
    if key not in _CACHE:
        _CACHE[key] = _build_nc(iters)
    return _CACHE[key]


def _host_constants():
    if "consts" not in _CACHE:
        import ml_dtypes
        ident = np.vstack([np.eye(M, dtype=np.float32)] * 2)
        identb = np.eye(128, dtype=np.float32).astype(ml_dtypes.bfloat16)
        triu = np.triu(np.ones((M, M), np.float32), k=1)
        ones64 = np.ones((M, 1), np.float32)
        ones128 = np.ones((128, 1), np.float32)
        _CACHE["consts"] = dict(ident=ident, identb=identb, triu=triu,
                                ones64=ones64, ones128=ones128)
    return _CACHE["consts"]


def kernel(x, W, label_batch=None, _want_time=False, _iters=1, **_ignored):
    import ml_dtypes
    x = np.asarray(x, dtype=np.float32)
    W = np.asarray(W, dtype=np.float32)
    nc = _get_nc(_iters)
    consts = _host_constants()

    wt = np.ascontiguousarray(
        W.T.reshape(2, 128, M)).astype(ml_dtypes.bfloat16)
    x_flat = x.reshape(B, C, N)

    in_maps = []
    for i in range(N_CORES):
        in_maps.append(dict(
            x=np.ascontiguousarray(x_flat[i * BS:(i + 1) * BS]),
            wt=wt, **consts))

    res = run_bass_kernel_spmd(nc, in_maps, list(range(N_CORES)))

    nq = np.concatenate(
        [res.results[i]["out_nq"].reshape(BS, C, N) for i in range(N_CORES)],
        axis=0)                                   # [B, C, N] unnormalized
    scal = np.stack([res.results[i]["scal"] for i in range(N_CORES)])  # [8,BS,2]
    summax = scal[:, :, 0].reshape(-1)            # [B]
    dis = scal[:, :, 1].reshape(-1)               # [B]

    # host: l2-normalize over C and reshape; losses
    nrm = np.sqrt(np.einsum("bcn,bcn->bn", nq, nq))
    np.maximum(nrm, 1e-12, out=nrm)
    out = (nq / nrm[:, None, :]).reshape(B, C, H, Wd).astype(np.float32)

    frob2 = float(np.dot(x.reshape(-1), x.reshape(-1)))
    compact = np.float32(
        (frob2 - 2.0 * float(summax.sum()) + B * N) / (B * N * C))
    dis_loss = np.float32(dis.mean() * 2.0 / (M * (M - 1)))

    if _want_time:
        return (out, compact, dis_loss), res
    return (out, compact, dis_loss)
